# revision 15
# baseline (speedup 1.0000x reference)
"""Multi-head attention (B=8, T=2048, C=256, H=4) on 8 NeuronCores.

Data-parallel over batch: core b computes batch element b end-to-end.

Per-core dataflow — everything runs "transposed" so the attention
contraction dims land on SBUF partitions and the big score matrices
never need transposing:

  xT   [C, T]      host-pretransposed, DMA'd straight into SBUF (bf16)
  qkT  [2C, T]     = w_qk @ xT + b_qk   (q/k for all heads; a head PAIR
                                         occupies the two 64-partition
                                         strips of each 128-row chunk)
  v    [T, H, 65]  = x @ w_v.T + b_v    (natural layout; bias AND the
                     ones column land via one K=1 matmul that seeds the
                     PSUM accumulator, so only a single copy remains)
  per (q-tile of 512 outer, head-pair inner), 16 groups of one k-chunk:
    scoresT[k,q] chunks via K=64 matmuls in PE row groups 0/64
    exp on ScalarE straight out of 2-bank PSUM groups (scale=1/8 fused)
    PV accumulates out2T[65, 512] in PSUM over all 16 k-chunks;
      row 64 = sum(exp) thanks to the ones column
    deferred normalization: unnormalized out2T is copied to yT, sumexp
      rows collected at 32-partition spacing; one batched
      reciprocal_approx_fast per q-tile, K=1 bf16 ones-matmul broadcasts
      1/sumexp across 64 partitions, one DVE multiply per head
    proj: out[t-chunk, :] = yT[:, t-chunk].T @ w_pT + b_p

The emission order is software-pipelined around the Scalar engine (the
128 exp instructions are the throughput floor): score matmuls run one
group ahead of the PV matmuls, and all projection / normalization /
stage-B / stage-C work is injected as "side work" between attention
groups with deadline-ordered placement so ScalarE never starves.

Dtypes: all matmuls bf16 (1 cyc/row); scores/PSUM stay fp32. Softmax
skips max-subtraction: logits are ~N(0, 1/3) so exp() is safely in
range.
"""

import numpy as np
import ml_dtypes

import concourse.bass as bass
import concourse.tile as tile
from concourse import bacc, mybir
from concourse.bass_utils import run_bass_kernel_spmd

B, T, C = 8, 2048, 256
H, HD = 4, 64
N_CORES = 8
F32 = mybir.dt.float32
F32R = mybir.dt.float32r
BF16 = mybir.dt.bfloat16

QT = 512                # q-tile (columns of scoresT per inner iteration)
NQT = T // QT           # 4
KC = T // 128           # 16 k-chunks of 128


def build_nc():
    nc = bacc.Bacc("TRN2", target_bir_lowering=False, debug=False,
                   num_devices=N_CORES)

    xT_ap = nc.dram_tensor("xT", [C, T], BF16, kind="ExternalInput").ap()
    wqk_ap = nc.dram_tensor("w_qkT", [C, 2 * C], BF16, kind="ExternalInput").ap()
    wv_ap = nc.dram_tensor("w_vT", [C, C], BF16, kind="ExternalInput").ap()
    wp_ap = nc.dram_tensor("w_pT", [C, C], BF16, kind="ExternalInput").ap()
    bqk_ap = nc.dram_tensor("b_qk", [4, 128], F32, kind="ExternalInput").ap()
    bvo_ap = nc.dram_tensor("b_vo", [H * (HD + 1)], BF16, kind="ExternalInput").ap()
    bp_ap = nc.dram_tensor("b_p", [C], F32, kind="ExternalInput").ap()
    out_ap = nc.dram_tensor("out", [T, C], F32, kind="ExternalOutput").ap()

    with tile.TileContext(nc) as tc:
        with (
            tc.tile_pool(name="consts", bufs=1) as consts,
            tc.tile_pool(name="xt", bufs=1) as xtp,
            tc.tile_pool(name="qkt", bufs=1) as qktp,
            tc.tile_pool(name="vsb", bufs=1) as vsbp,
            tc.tile_pool(name="expp", bufs=6) as expp,
            tc.tile_pool(name="yt", bufs=1) as ytp,
            tc.tile_pool(name="ostage", bufs=4) as ostage,
            tc.tile_pool(name="small", bufs=4) as small,
            tc.tile_pool(name="scps", bufs=3, space="PSUM") as scps,
            tc.tile_pool(name="o2ps", bufs=1, space="PSUM") as o2ps,
        ):
            # ---- x loads first: everything downstream hangs off them ----
            xt = [xtp.tile([128, T], BF16, tag=f"xt{c}", name=f"xt{c}") for c in range(2)]
            for c in range(2):
                nc.gpsimd.dma_start(xt[c][:, 0:QT], xT_ap[128 * c:128 * (c + 1), 0:QT])
            for c in range(2):
                nc.gpsimd.dma_start(xt[c][:, QT:T], xT_ap[128 * c:128 * (c + 1), QT:T])

            # ---- weights / biases on the Sync DMA queue -----------------
            w_qk = [consts.tile([128, 2 * C], BF16, tag=f"wqk{c}", name=f"wqk{c}") for c in range(2)]
            for c in range(2):
                nc.sync.dma_start(w_qk[c][:], wqk_ap[128 * c:128 * (c + 1), :])
            w_v = [consts.tile([128, C], BF16, tag=f"wv{c}", name=f"wv{c}") for c in range(2)]
            for c in range(2):
                nc.sync.dma_start(w_v[c][:], wv_ap[128 * c:128 * (c + 1), :])
            w_p = [consts.tile([128, C], BF16, tag=f"wp{c}", name=f"wp{c}") for c in range(2)]
            for c in range(2):
                nc.sync.dma_start(w_p[c][:], wp_ap[128 * c:128 * (c + 1), :])

            b_qk = consts.tile([128, 4], F32, tag="bqk")
            nc.sync.dma_start(b_qk[:], bqk_ap.rearrange("c p -> p c"))
            b_vo = consts.tile([1, H * (HD + 1)], BF16, tag="bvo")
            bvo_row = bass.AP(tensor=bvo_ap.tensor, offset=bvo_ap.offset,
                              ap=[[0, 1]] + list(bvo_ap.ap))
            nc.sync.dma_start(b_vo[:], bvo_row)
            b_p = consts.tile([128, C], F32, tag="bp")
            bp_bc = bass.AP(tensor=bp_ap.tensor, offset=bp_ap.offset,
                            ap=[[0, 128]] + list(bp_ap.ap))
            nc.sync.dma_start(b_p[:], bp_bc)

            ones_b = consts.tile([97, 128], BF16, tag="ones_b")
            nc.vector.memset(ones_b[:], 1.0)

            # ---- persistent SBUF state ----------------------------------
            qkt = [qktp.tile([128, T], BF16, tag=f"qkt{m}", name=f"qkt{m}") for m in range(4)]
            vsb = [vsbp.tile([128, H, HD + 1], BF16, tag=f"v{tt}", name=f"v{tt}") for tt in range(KC)]
            yt = [ytp.tile([128, T], BF16, tag=f"yt{hp}", name=f"yt{hp}") for hp in range(2)]

            # ---- unit builders ------------------------------------------
            def stage_b(n, m):
                # qkT[m][:, 512n:512(n+1)] = w_qk[:, 128m block].T @ xT + b
                ps = scps.tile([128, QT], F32, tag="sc", name=f"bps{m}")
                for c in range(2):
                    nc.tensor.matmul(
                        ps[:], w_qk[c][:, 128 * m:128 * (m + 1)],
                        xt[c][:, QT * n:QT * (n + 1)],
                        start=(c == 0), stop=(c == 1))
                nc.vector.tensor_scalar_add(
                    qkt[m][:, QT * n:QT * (n + 1)], ps[:], b_qk[:, m:m + 1])

            def stage_c(tt):
                # v[t-chunk] = x @ w_v.T (+bias and ones column via a K=1
                # matmul that seeds the accumulator)
                ps = scps.tile([128, H, HD + 1], F32, tag="sc", name="vps")
                nc.tensor.matmul(ps[:], ones_b[0:1, :], b_vo[:],
                                 start=True, stop=False, skip_group_check=True)
                for c in range(2):
                    nc.tensor.matmul(
                        ps[:, :, 0:HD], xt[c][:, 128 * tt:128 * (tt + 1)], w_v[c][:],
                        start=False, stop=(c == 1), skip_group_check=True)
                nc.vector.tensor_copy(vsb[tt][:], ps[:])

            # per-(qt, hp) normalization state
            se_t = [[None] * 2 for _ in range(NQT)]
            rec_t = [[None] * 2 for _ in range(NQT)]

            def ytse(qt, hp, h, o2h):
                # unnormalized head output to SBUF (DVE); sumexp row on the
                # Scalar engine, which is otherwise idle at phase boundaries
                nc.vector.tensor_copy(
                    yt[hp][64 * h:64 * (h + 1), QT * qt:QT * (qt + 1)],
                    o2h[0:HD, :])
                nc.scalar.copy(
                    se_t[qt][hp][32 * h:32 * h + 1, :], o2h[HD:HD + 1, :])

            def recip(qt, hp):
                rec_f = small.tile([33, QT], F32, tag="rec_f")
                nc.vector.reciprocal_approx_fast(rec_f[:], se_t[qt][hp][:])
                rec_t[qt][hp] = small.tile([33, QT], BF16, tag="rec",
                                           name=f"rec{qt}_{hp}")
                nc.vector.tensor_copy(rec_t[qt][hp][:], rec_f[:])

            def norm(qt, hp, h):
                # broadcast 1/sumexp across 64 partitions (bf16 K=1 matmul)
                p = 32 * h
                bc = scps.tile([HD, QT], F32, tag="sc", name=f"bc{h}")
                nc.tensor.matmul(bc[:], ones_b[p:p + 1, 0:HD],
                                 rec_t[qt][hp][p:p + 1, :],
                                 start=True, stop=True, tile_position=(p, 0))
                ys = yt[hp][64 * h:64 * (h + 1), QT * qt:QT * (qt + 1)]
                nc.vector.tensor_mul(ys, ys, bc[:])

            def proj(tt):
                ps = scps.tile([128, C], F32, tag="sc", name="pps")
                for c in range(2):
                    nc.tensor.matmul(
                        ps[:], yt[c][:, 128 * tt:128 * (tt + 1)], w_p[c][:],
                        start=(c == 0), stop=(c == 1))
                ost = ostage.tile([128, C], F32, tag="ost")
                nc.vector.tensor_add(ost[:], ps[:], b_p[:])
                nc.sync.dma_start(out_ap[128 * tt:128 * (tt + 1), :], ost[:])

            # ---- side-work schedule -------------------------------------
            # Each (qt, hp) phase has 16 attention groups; side[g] is a list
            # of closures emitted just before group g's score matmuls.
            def hp0_side(qt):
                s = [[] for _ in range(KC)]
                if qt == 0:
                    # stage C chunk tt must land before group tt (PV reads
                    # vsb[tt]); kT head-pair 0 column block n before group 4n.
                    place = {0: [lambda: stage_c(2), lambda: stage_c(3)],
                             1: [lambda: stage_c(4), lambda: stage_b(1, 2)],
                             2: [lambda: stage_c(5), lambda: stage_c(6)],
                             3: [lambda: stage_c(7), lambda: stage_b(2, 2)],
                             4: [lambda: stage_c(8), lambda: stage_c(9)],
                             5: [lambda: stage_c(10), lambda: stage_b(3, 2)],
                             6: [lambda: stage_c(11), lambda: stage_c(12)],
                             7: [lambda: stage_c(13), lambda: stage_b(0, 1)],
                             8: [lambda: stage_c(14)],
                             9: [lambda: stage_c(15), lambda: stage_b(0, 3)]}
                    for g, items in place.items():
                        s[g].extend(items)
                else:
                    # previous q-tile's hp1 normalization + projection + store
                    pq = qt - 1
                    s[0].append(lambda: recip(pq, 1))
                    s[1].append(lambda: norm(pq, 1, 0))
                    s[2].append(lambda: norm(pq, 1, 1))
                    for g in range(4):
                        s[g + 3].append(lambda g=g: proj(4 * pq + g))
                    if qt < NQT - 1:
                        s[8].append(lambda: stage_b(qt + 1, 0))
                return s

            def hp1_side(qt):
                # hp0's normalization runs here, overlapped with hp1 groups
                s = [[] for _ in range(KC)]
                s[0].append(lambda: recip(qt, 0))
                s[1].append(lambda: norm(qt, 0, 0))
                s[2].append(lambda: norm(qt, 0, 1))
                if qt == 0:
                    s[3].append(lambda: stage_b(1, 3))
                    s[4].append(lambda: stage_b(2, 3))
                    s[5].append(lambda: stage_b(3, 3))
                    s[6].append(lambda: stage_b(1, 0))
                    s[8].append(lambda: stage_b(1, 1))
                elif qt < NQT - 1:
                    s[3].append(lambda: stage_b(qt + 1, 1))
                return s

            # ---- prologue: minimum work before the first exp ------------
            stage_b(0, 0)   # qT head-pair 0, cols 0:512
            stage_b(0, 2)   # kT head-pair 0, cols 0:512
            stage_c(0)
            stage_c(1)

            # ---- attention: 8 phases x 16 groups, PV deferred 2 groups --
            DEFER = 2
            for qt in range(NQT):
                for hp in range(2):
                    se_t[qt][hp] = small.tile([33, QT], F32, tag=f"se{hp}",
                                              name=f"se{qt}_{hp}")
                    side = hp0_side(qt) if hp == 0 else hp1_side(qt)
                    qT = qkt[hp]
                    kT = qkt[hp + 2]
                    o2 = [o2ps.tile([HD + 1, QT], F32, tag=f"o2{h}", name=f"o2{h}")
                          for h in range(2)]

                    def pv(i, ex):
                        for h in range(2):
                            nc.tensor.matmul(
                                o2[h][:], vsb[i][:, 2 * hp + h, :], ex[:, h, :],
                                start=(i == 0), stop=(i == KC - 1))

                    pend = []   # (i, ex) awaiting PV matmuls
                    for i in range(KC):
                        for work in side[i]:
                            work()
                        sc = scps.tile([128, 2, QT], F32, tag="sc")
                        for h in range(2):
                            nc.tensor.matmul(
                                sc[:, h, :],
                                kT[64 * h:64 * (h + 1), 128 * i:128 * (i + 1)],
                                qT[64 * h:64 * (h + 1), QT * qt:QT * (qt + 1)],
                                start=True, stop=True)
                        ex = expp.tile([128, 2, QT], BF16, tag="ex")
                        nc.scalar.activation(
                            ex[:], sc[:],
                            mybir.ActivationFunctionType.Exp,
                            bias=0.0, scale=float(HD) ** -0.5)
                        pend.append((i, ex))
                        if len(pend) > DEFER:
                            pv(*pend.pop(0))
                    for item in pend:
                        pv(*item)
                    for h in range(2):
                        ytse(qt, hp, h, o2[h])

            # ---- epilogue: last q-tile hp1 normalization + projection ---
            recip(NQT - 1, 1)
            for h in range(2):
                norm(NQT - 1, 1, h)
            for tt in range(4 * (NQT - 1), 4 * NQT):
                proj(tt)
    nc.compile()
    return nc


_NC_CACHE = []


def _get_nc():
    if not _NC_CACHE:
        _NC_CACHE.append(build_nc())
    return _NC_CACHE[0]


def make_in_maps(x, w_qkv, b_qkv, w_proj, b_proj):
    b_v = np.asarray(b_qkv[2 * C:], dtype=np.float32).reshape(H, HD)
    b_vo = np.concatenate([b_v, np.ones((H, 1), np.float32)], axis=1)
    shared = {
        "w_qkT": np.ascontiguousarray(w_qkv[:2 * C].T.astype(ml_dtypes.bfloat16)),
        "w_vT": np.ascontiguousarray(w_qkv[2 * C:].T.astype(ml_dtypes.bfloat16)),
        "w_pT": np.ascontiguousarray(w_proj.T.astype(ml_dtypes.bfloat16)),
        "b_qk": np.ascontiguousarray(b_qkv[:2 * C].reshape(4, 128), dtype=np.float32),
        "b_vo": np.ascontiguousarray(b_vo.reshape(-1).astype(ml_dtypes.bfloat16)),
        "b_p": np.ascontiguousarray(b_proj, dtype=np.float32),
    }
    return [dict(shared,
                 xT=np.ascontiguousarray(x[b].T.astype(ml_dtypes.bfloat16)))
            for b in range(B)]


def run(x, w_qkv, b_qkv, w_proj, b_proj, trace=False):
    nc = _get_nc()
    in_maps = make_in_maps(np.asarray(x), np.asarray(w_qkv), np.asarray(b_qkv),
                           np.asarray(w_proj), np.asarray(b_proj))
    res = run_bass_kernel_spmd(nc, in_maps, list(range(N_CORES)), trace=trace)
    out = np.stack([res.results[b]["out"] for b in range(B)])
    return out, res


def kernel(x, w_qkv, b_qkv, w_proj, b_proj):
    out, _ = run(x, w_qkv, b_qkv, w_proj, b_proj, trace=False)
    return out
